# revision 12
# baseline (speedup 1.0000x reference)
"""Single-head attention (B=4, S=4096, F=H=1024) on 8 TRN2 NeuronCores.

Sharding: core = 2*b + h owns batch b, sequence-half h (rows h*2048 ..
(h+1)*2048). Each core projects K/Q/V only for its OWN 2048 rows, then the
two cores of a batch exchange K^T and V with pair-wise AllGathers (2-core
replica groups), slab-granular so comm hides behind compute.

Precision scheme (validated offline against the seeded reference inputs,
rel-err 1.2e-2 < 2e-2 gate):
  - QKV projections in bf16 (fp8 projections would push score/V noise over
    the error budget).
  - Q^T, K^T, V stored as e4m3 fp8 at scale 16; the scores matmul and the
    attention*V matmul run in fp8 DoubleRow perf mode (256-row contraction
    per instruction, 2x bf16 FLOP rate).
  - P = exp(s) has ~2.7% fp8 quantization noise, too much to hit the error
    budget directly; instead store P' = (P - C)*16 in fp8 (C ~ E[P]) -- a
    3x smaller quantization target -- and add back the rank-1 correction
    C * colsum(V) via a K=1 bf16 matmul opening each output PSUM group.
    colsum(V) over the full key range comes from the *unquantized* V path
    (colsum(x) @ Wv + S*bv, via a DVE free-dim reduce + 16 small matmuls +
    a tiny pair AllReduce), which also cancels the mean component of V's
    fp8 quantization error. Denominator = sum_k P'/256 + S*C via a
    DoubleRow ones-column matmul and a scalar add before the reciprocal.

Per-core math:
  x^T (own half) passed pre-transposed bf16 from host: [F=1024, 2048].
  K^T[h,s] = fp8(16*(sum_f Wk[f,h] x^T[f,s] + bk))   (activation scale+bias)
  Q^T likewise, resident in SBUF. V[s,h] = fp8(16*(x@Wv + bv)).
  S^T[k,q] = sum_h K^T[h,k] Q^T[h,q]  (fp8 DoubleRow, 4 instr per 512 q)
  P = exp(S^T/8192) -> bf16;  P' = (P - C)*16 -> fp8 (DVE 2-op, pair-packed)
  out[q,:] = (P'^T V / 256 + C colsum(V)) / (sum_k P'/256 + S*C)
"""

import numpy as np
import ml_dtypes

# bass_utils' trace path imports antenv.axon_hooks, which some images lack;
# provide a no-op fallback so an externally-set BASS_TRACE cannot crash us.
try:
    import antenv.axon_hooks  # noqa: F401
except Exception:  # pragma: no cover
    try:
        import sys as _sys
        import types as _types

        import antenv as _antenv

        _m = _types.ModuleType("antenv.axon_hooks")
        _m.set_axon_ntff_profile_hook = lambda h: None
        _m.get_axon_ntff_profile_hook = lambda: None
        _sys.modules["antenv.axon_hooks"] = _m
        _antenv.axon_hooks = _m
    except Exception:
        pass

import concourse.bass as bass  # noqa: F401  (registers engine types)
import concourse.mybir as mybir
import concourse.tile as tile
from concourse import bacc
from concourse.bass_utils import run_bass_kernel_spmd

BF16 = mybir.dt.bfloat16
F8 = mybir.dt.float8e4
F32 = mybir.dt.float32
AF = mybir.ActivationFunctionType
DR = mybir.MatmulPerfMode.DoubleRow

B, S, F, H = 4, 4096, 1024, 1024
QH = S // 2  # rows owned per core
FC = F // 128  # 8 feature chunks
HC = H // 128  # 8 hidden chunks
KC = S // 128  # 32 key chunks (full sequence)
N_CORES = 8
QS = 16.0  # fp8 scale for q/k/v/p'
CMEAN = 1.0568  # ~ E[exp(score)] for these inputs; any value is *correct*
EXP_SCALE = 1.0 / (32.0 * QS * QS)  # scores psum carries 256x
DEN_ADD = QS * QS * S * CMEAN  # add to osum psum before reciprocal
PAIRS = [[0, 1], [2, 3], [4, 5], [6, 7]]

# key-chunk processing order: slab-0-dependent chunks (cols 0:1024 of each
# half) first, then slab-1 chunks.  k = half*16 + kk, slab = kk//8.
# Adjacent pairs share a gather-group g = slab*2+half and j parity (even,odd),
# which is exactly the DoubleRow pairing used in the AV matmul.
K_ORDER = (
    list(range(0, 8)) + list(range(16, 24)) + list(range(8, 16)) + list(range(24, 32))
)

_NC_CACHE = None


def _build_nc():
    nc = bacc.Bacc("TRN2", target_bir_lowering=False, debug=False)

    xt_ext = nc.declare_dram_parameter("xt", [F, QH], BF16, isOutput=False)
    wq_ext = nc.declare_dram_parameter("wq", [F, H], BF16, isOutput=False)
    wk_ext = nc.declare_dram_parameter("wk", [F, H], BF16, isOutput=False)
    wv_ext = nc.declare_dram_parameter("wv", [F, H], BF16, isOutput=False)
    bqt_ext = nc.declare_dram_parameter("bqt", [128, HC], F32, isOutput=False)
    bkt_ext = nc.declare_dram_parameter("bkt", [128, HC], F32, isOutput=False)
    bv_ext = nc.declare_dram_parameter("bv", [1, H], BF16, isOutput=False)
    bvr_ext = nc.declare_dram_parameter("bvr", [1, H], F32, isOutput=False)
    out_ext = nc.declare_dram_parameter("out", [QH, H], F32, isOutput=True)

    xt_v = xt_ext[:].rearrange("(c p) s -> p c s", p=128)
    wq_v = wq_ext[:].rearrange("(c p) h -> p c h", p=128)
    wk_v = wk_ext[:].rearrange("(c p) h -> p c h", p=128)
    wv_v = wv_ext[:].rearrange("(c p) h -> p c h", p=128)

    with tile.TileContext(nc) as tc:
        with (
            tc.tile_pool(name="const", bufs=1) as constp,
            tc.tile_pool(name="qtres", bufs=1) as qtpool,
            tc.tile_pool(name="spill", bufs=1, space="DRAM") as dramp,
        ):
            ones_lhs = constp.tile([1, 128], BF16, tag="ones_lhs", name="ones_lhs")
            nc.vector.memset(ones_lhs[:], 1.0)
            cones8 = constp.tile([128, 2, 1], F8, tag="cones8", name="cones8")
            nc.vector.memset(cones8[:], QS)
            bqt = constp.tile([128, HC], F32, tag="bqt", name="bqt")
            bkt = constp.tile([128, HC], F32, tag="bkt", name="bkt")
            bv_sb = constp.tile([1, H], BF16, tag="bv", name="bv_sb")
            bvr_sb = constp.tile([1, H], F32, tag="bvr", name="bvr_sb")
            csrow = constp.tile([1, H], BF16, tag="csrow", name="csrow")

            # per-slab own spills + gathered pair buffers (plain Local DRAM)
            kt_own = [
                dramp.tile([HC, 128, 1024], F8, tag=f"kto{s}", name=f"kt_own{s}")
                for s in range(2)
            ]
            v_own = [
                dramp.tile([1024, H], F8, tag=f"vo{s}", name=f"v_own{s}")
                for s in range(2)
            ]
            kt_gath = [
                dramp.tile([2, HC, 128, 1024], F8, tag=f"ktg{s}", name=f"kt_gath{s}")
                for s in range(2)
            ]
            v_gath = [
                dramp.tile([2, 1024, H], F8, tag=f"vg{s}", name=f"v_gath{s}")
                for s in range(2)
            ]
            cs_own = dramp.tile([1, H], F32, tag="cso", name="cs_own")
            cs_full = dramp.tile([1, H], F32, tag="csf", name="cs_full")

            qt_res = qtpool.tile([128, HC, QH], F8, tag="qtres", name="qt_res")
            # g = slab*2 + half; prefetched during phase A's Q projection
            vbig = [
                qtpool.tile([128, 8, H], F8, tag=f"vb{g}", name=f"vbig{g}")
                for g in range(4)
            ]
            ktbig = [
                qtpool.tile([128, HC, 1024], F8, tag=f"kb{g}", name=f"ktbig{g}")
                for g in range(4)
            ]

            def pair_gather(dst, src):
                nc.gpsimd.collective_compute(
                    "AllGather", mybir.AluOpType.bypass, replica_groups=PAIRS,
                    ins=[src.opt()], outs=[dst.opt()],
                )

            # ---------- Phase A: own-half projections in one x^T pass ----------
            with (
                tc.tile_pool(name="wp", bufs=1) as wp,
                tc.tile_pool(name="xp", bufs=2) as xp,
                tc.tile_pool(name="stage", bufs=2) as stp,
                tc.tile_pool(name="csp", bufs=1) as csp,
                tc.tile_pool(name="psA", bufs=4, space="PSUM") as psA,
                tc.tile_pool(name="psC", bufs=2, space="PSUM") as psC,
            ):
                wk_sb = wp.tile([128, FC, H], BF16, tag="wk", name="wk_sb")
                wq_sb = wp.tile([128, FC, H], BF16, tag="wq", name="wq_sb")
                wv_sb = wp.tile([128, FC, H], BF16, tag="wv", name="wv_sb")
                nc.sync.dma_start(wk_sb[:, :, 0:512], wk_v[:, :, 0:512])

                xts_l = []
                for sp in range(QH // 1024):  # 1024-column slabs of own x^T
                    xts = xp.tile([128, FC, 1024], BF16, tag="xts", name=f"xts{sp}")
                    xts_l.append(xts)
                    base = sp * 1024
                    nc.sync.dma_start(xts[:, :, 0:512], xt_v[:, :, base : base + 512])
                    if sp == 0:
                        nc.sync.dma_start(wk_sb[:, :, 512:1024], wk_v[:, :, 512:1024])
                        nc.sync.dma_start(bkt[:], bkt_ext[:])
                        nc.sync.dma_start(bqt[:], bqt_ext[:])
                        nc.sync.dma_start(bv_sb[:], bv_ext[:])
                        nc.sync.dma_start(bvr_sb[:], bvr_ext[:])
                    nc.sync.dma_start(
                        xts[:, :, 512:1024], xt_v[:, :, base + 512 : base + 1024]
                    )
                # wv is first needed after both K slabs (~55us in), wq after V
                # (~110us): issue after the x^T slabs so they don't contend
                # with the startup-critical loads.
                nc.sync.dma_start(wv_sb[:], wv_v)
                nc.sync.dma_start(wq_sb[:], wq_v)

                # K^T both slabs first, so both pair-gathers start early.
                # Key-half-major order: the second x^T half-slab DMA streams
                # behind the first half's 13us of matmuls instead of stalling
                # the per-hh ps0/ps1 interleave at startup.
                for sp in range(QH // 1024):
                    xts = xts_l[sp]
                    for kh in range(2):
                        kq = slice(kh * 512, kh * 512 + 512)
                        kst = stp.tile(
                            [128, HC, 512], F8, tag="kst", name=f"ks{sp}_{kh}"
                        )
                        for hh in range(HC):
                            ps = psA.tile(
                                [128, 512], F32, tag="psA", name=f"pk_{sp}_{kh}_{hh}"
                            )
                            for f in range(FC):
                                nc.tensor.matmul(
                                    ps[:], wk_sb[:, f, hh * 128 : (hh + 1) * 128],
                                    xts[:, f, kq],
                                    start=(f == 0), stop=(f == FC - 1),
                                )
                            nc.scalar.activation(
                                kst[:, hh, :], ps[:], AF.Identity,
                                bias=bkt[:, hh : hh + 1], scale=QS,
                            )
                        nc.sync.dma_start(
                            kt_own[sp][:, :, kq].rearrange("c p q -> p c q"), kst[:]
                        )
                    pair_gather(kt_gath[sp], kt_own[sp])

                # colsum(V_true) over own half: DVE reduce of x^T + 16 small
                # matmuls against Wv + host-precomputed 2048*bv, then pair
                # AllReduce to cover the partner half. Placed before V proj so
                # the AllReduce queues right after the K^T gathers -- the AV
                # correction matmul consumes csrow at the start of phase B.
                cx0 = csp.tile([128, FC, 1], F32, tag="cx0", name="cx0")
                cx1 = csp.tile([128, FC, 1], F32, tag="cx1", name="cx1")
                cx16 = csp.tile([128, FC, 1], BF16, tag="cx16", name="cx16")
                nc.vector.tensor_reduce(
                    cx0[:], xts_l[0][:], mybir.AxisListType.X, mybir.AluOpType.add
                )
                nc.vector.tensor_reduce(
                    cx1[:], xts_l[1][:], mybir.AxisListType.X, mybir.AluOpType.add
                )
                nc.vector.tensor_tensor(
                    cx16[:], cx0[:], cx1[:], mybir.AluOpType.add
                )
                csO = csp.tile([1, H], F32, tag="csO", name="csO")
                for half in range(2):
                    pcs = psC.tile([1, 512], F32, tag="psC", name=f"pcs{half}")
                    for f in range(FC):
                        nc.tensor.matmul(
                            pcs[:], cx16[:, f, :],
                            wv_sb[:, f, half * 512 : half * 512 + 512],
                            start=(f == 0), stop=(f == FC - 1),
                        )
                    nc.vector.tensor_tensor(
                        csO[:, half * 512 : half * 512 + 512], pcs[:],
                        bvr_sb[:, half * 512 : half * 512 + 512], mybir.AluOpType.add,
                    )
                nc.sync.dma_start(cs_own[:], csO[:])
                nc.gpsimd.collective_compute(
                    "AllReduce", mybir.AluOpType.add, replica_groups=PAIRS,
                    ins=[cs_own[:].opt()], outs=[cs_full[:].opt()],
                )
                csF = csp.tile([1, H], F32, tag="csF", name="csF")
                nc.sync.dma_start(csF[:], cs_full[:])
                # csrow = 256*C*colsum, consumed by the K=1 bf16 corr matmul
                nc.vector.tensor_scalar_mul(csrow[:], csF[:], QS * QS * CMEAN)

                # V both slabs (fp8 out at scale QS; bias via ones-row matmul)
                for sp in range(QH // 1024):
                    xts = xts_l[sp]
                    vst = stp.tile([128, 8, H], F8, tag="vst", bufs=2, name=f"vst{sp}")
                    for sc in range(8):
                        ps0 = psA.tile([128, 512], F32, tag="psA", name=f"pv0_{sp}_{sc}")
                        ps1 = psA.tile([128, 512], F32, tag="psA", name=f"pv1_{sp}_{sc}")
                        for f in range(FC):
                            lhs = xts[:, f, sc * 128 : (sc + 1) * 128]
                            nc.tensor.matmul(
                                ps0[:], lhs, wv_sb[:, f, 0:512],
                                start=(f == 0), stop=False,
                            )
                            nc.tensor.matmul(
                                ps1[:], lhs, wv_sb[:, f, 512:1024],
                                start=(f == 0), stop=False,
                            )
                        nc.tensor.matmul(
                            ps0[:], ones_lhs[:], bv_sb[:, 0:512], start=False, stop=True
                        )
                        nc.tensor.matmul(
                            ps1[:], ones_lhs[:], bv_sb[:, 512:1024], start=False, stop=True
                        )
                        nc.vector.tensor_scalar_mul(vst[:, sc, 0:512], ps0[:], QS)
                        nc.vector.tensor_scalar_mul(vst[:, sc, 512:1024], ps1[:], QS)
                    nc.sync.dma_start(
                        v_own[sp][:].rearrange("(c p) h -> p c h", p=128), vst[:]
                    )
                    pair_gather(v_gath[sp], v_own[sp])

                # prefetch gathered K^T/V into phase-B residents while Q proj
                # still runs (kvpool lives in the outer scope)
                for g in range(4):
                    slab, half = g // 2, g % 2
                    nc.sync.dma_start(
                        ktbig[g][:],
                        kt_gath[slab][half].rearrange("c p k -> p c k"),
                    )
                    nc.sync.dma_start(
                        vbig[g][:],
                        v_gath[slab][half].rearrange("(c p) h -> p c h", p=128),
                    )

                # Q^T both slabs -> resident SBUF (fp8 at scale QS)
                for sp in range(QH // 1024):
                    xts = xts_l[sp]
                    base = sp * 1024
                    for hh in range(HC):
                        ps0 = psA.tile([128, 512], F32, tag="psA", name=f"pq0_{sp}_{hh}")
                        ps1 = psA.tile([128, 512], F32, tag="psA", name=f"pq1_{sp}_{hh}")
                        for f in range(FC):
                            lhs = wq_sb[:, f, hh * 128 : (hh + 1) * 128]
                            nc.tensor.matmul(
                                ps0[:], lhs, xts[:, f, 0:512],
                                start=(f == 0), stop=(f == FC - 1),
                            )
                            nc.tensor.matmul(
                                ps1[:], lhs, xts[:, f, 512:1024],
                                start=(f == 0), stop=(f == FC - 1),
                            )
                        bias = bqt[:, hh : hh + 1]
                        nc.scalar.activation(
                            qt_res[:, hh, base : base + 512], ps0[:],
                            AF.Identity, bias=bias, scale=QS,
                        )
                        nc.scalar.activation(
                            qt_res[:, hh, base + 512 : base + 1024], ps1[:],
                            AF.Identity, bias=bias, scale=QS,
                        )

            # ---------- Phase B: attention, 1024 query rows per tile ----------
            with (
                tc.tile_pool(name="expp", bufs=2) as expp,
                tc.tile_pool(name="pexpp", bufs=4) as pexpp,
                tc.tile_pool(name="obp", bufs=3) as obp,
                tc.tile_pool(name="psS", bufs=3, space="PSUM") as psS,
                tc.tile_pool(name="psO", bufs=2, space="PSUM") as psO,
            ):
                for qt in range(QH // 1024):
                    qbase = qt * 1024
                    # scores + exp + P' quant, one key-chunk at a time
                    exps = {}
                    for pos, k in enumerate(K_ORDER):
                        i, par = pos // 2, pos % 2
                        half, kk = k // 16, k % 16
                        slab, kk8 = kk // 8, kk % 8
                        g = slab * 2 + half
                        if par == 0:
                            exps[i] = expp.tile(
                                [128, 2, 1024], F8, tag=f"e{i}", name=f"e{qt}_{i}"
                            )
                        ps0 = psS.tile([128, 512], F32, tag="psS", name=f"pS0_{qt}_{k}")
                        ps1 = psS.tile([128, 512], F32, tag="psS", name=f"pS1_{qt}_{k}")
                        kslice = slice(kk8 * 128, (kk8 + 1) * 128)
                        for hp in range(HC // 2):
                            nc.tensor.matmul(
                                ps0[:], ktbig[g][:, 2 * hp : 2 * hp + 2, kslice],
                                qt_res[:, 2 * hp : 2 * hp + 2, qbase : qbase + 512],
                                start=(hp == 0), stop=(hp == HC // 2 - 1),
                                perf_mode=DR,
                            )
                        for hp in range(HC // 2):
                            nc.tensor.matmul(
                                ps1[:], ktbig[g][:, 2 * hp : 2 * hp + 2, kslice],
                                qt_res[:, 2 * hp : 2 * hp + 2, qbase + 512 : qbase + 1024],
                                start=(hp == 0), stop=(hp == HC // 2 - 1),
                                perf_mode=DR,
                            )
                        pexp = pexpp.tile([128, 1024], BF16, tag="pexp", name=f"px{qt}_{k}")
                        nc.scalar.activation(pexp[:, 0:512], ps0[:], AF.Exp, scale=EXP_SCALE)
                        nc.scalar.activation(pexp[:, 512:1024], ps1[:], AF.Exp, scale=EXP_SCALE)
                        nc.vector.tensor_scalar(
                            exps[i][:, par, :], pexp[:], -CMEAN, QS,
                            mybir.AluOpType.add, mybir.AluOpType.mult,
                        )
                    # AV: fp8 DoubleRow over key-chunk pairs; the K=1 bf16
                    # corr matmul opens each group with C*colsum(V)*256
                    for q1 in range(8):
                        qo = q1 * 128
                        o0 = psO.tile([128, 512], F32, tag="o0", name=f"o0_{qt}_{q1}")
                        o1 = psO.tile([128, 512], F32, tag="o1", name=f"o1_{qt}_{q1}")
                        osum = psO.tile(
                            [128, 1], F32, tag="osum", bufs=1, name=f"os{qt}_{q1}"
                        )
                        nc.tensor.matmul(
                            o0[:], ones_lhs[:], csrow[:, 0:512], start=True, stop=False
                        )
                        nc.tensor.matmul(
                            o1[:], ones_lhs[:], csrow[:, 512:1024], start=True, stop=False
                        )
                        for t in range(16):
                            k0 = K_ORDER[2 * t]
                            half, kk = k0 // 16, k0 % 16
                            slab, j = kk // 8, kk % 8
                            g = slab * 2 + half
                            lhs = exps[t][:, :, qo : qo + 128]
                            first, last = t == 0, t == 15
                            nc.tensor.matmul(
                                osum[:], lhs, cones8[:], start=first, stop=last,
                                perf_mode=DR,
                            )
                            nc.tensor.matmul(
                                o0[:], lhs, vbig[g][:, j : j + 2, 0:512],
                                start=False, stop=last, perf_mode=DR,
                            )
                            nc.tensor.matmul(
                                o1[:], lhs, vbig[g][:, j : j + 2, 512:1024],
                                start=False, stop=last, perf_mode=DR,
                            )
                        den = obp.tile([128, 1], F32, tag="den", name=f"dn{qt}_{q1}")
                        nc.vector.tensor_scalar_add(den[:], osum[:], DEN_ADD)
                        recip = obp.tile([128, 1], F32, tag="recip", name=f"rc{qt}_{q1}")
                        nc.vector.reciprocal(recip[:], den[:])
                        outsb = obp.tile([128, H], F32, tag="outsb", name=f"ou{qt}_{q1}")
                        row = qbase + qo
                        # normalize halves on different engines so the tail
                        # q1's two halves finish in parallel
                        nc.scalar.activation(
                            outsb[:, 0:512], o0[:], AF.Identity, scale=recip[:]
                        )
                        nc.sync.dma_start(
                            out_ext[row : row + 128, 0:512], outsb[:, 0:512]
                        )
                        nc.vector.tensor_scalar_mul(outsb[:, 512:1024], o1[:], recip[:])
                        nc.sync.dma_start(
                            out_ext[row : row + 128, 512:1024], outsb[:, 512:1024]
                        )

    nc.compile()
    return nc


def _get_nc():
    global _NC_CACHE
    if _NC_CACHE is None:
        _NC_CACHE = _build_nc()
    return _NC_CACHE


def _make_in_maps(x, Wq, bq, Wk, bk, Wv, bv):
    bf16 = ml_dtypes.bfloat16
    wq_b = np.asarray(Wq, np.float32).astype(bf16)
    wk_b = np.asarray(Wk, np.float32).astype(bf16)
    wv_b = np.asarray(Wv, np.float32).astype(bf16)
    # activation computes f(scale*x + bias) with scale=QS, so pre-scale biases
    bqt = np.ascontiguousarray(QS * np.asarray(bq, np.float32).reshape(HC, 128).T)
    bkt = np.ascontiguousarray(QS * np.asarray(bk, np.float32).reshape(HC, 128).T)
    bv_b = np.asarray(bv, np.float32).astype(bf16).reshape(1, H)
    bvr = (QH * np.asarray(bv, np.float32)).reshape(1, H)
    x = np.asarray(x, np.float32)
    in_maps = []
    for core in range(N_CORES):
        b, h = core // 2, core % 2
        xt = np.ascontiguousarray(x[b, h * QH : (h + 1) * QH].T).astype(bf16)
        in_maps.append(
            {
                "xt": xt,
                "wq": wq_b,
                "wk": wk_b,
                "wv": wv_b,
                "bqt": bqt,
                "bkt": bkt,
                "bv": bv_b,
                "bvr": bvr,
            }
        )
    return in_maps


def run_on_hw(inputs, trace=False, tmpdir=None):
    """Returns (full_output, BassKernelResults)."""
    nc = _get_nc()
    in_maps = _make_in_maps(**inputs)
    res = run_bass_kernel_spmd(
        nc, in_maps, core_ids=list(range(N_CORES)), trace=trace, tmpdir=tmpdir
    )
    out = np.empty((B, S, H), np.float32)
    for core in range(N_CORES):
        b, h = core // 2, core % 2
        out[b, h * QH : (h + 1) * QH] = res.results[core]["out"]
    return out, res


def kernel(x, Wq, bq, Wk, bk, Wv, bv):
    out, _ = run_on_hw(
        {"x": x, "Wq": Wq, "bq": bq, "Wk": Wk, "bk": bk, "Wv": Wv, "bv": bv}
    )
    return out


# revision 14
# speedup vs baseline: 1.1977x; 1.1977x over previous
"""Single-head attention (B=4, S=4096, F=H=1024) on 8 TRN2 NeuronCores.

Sharding: core = 2*b + h owns batch b, sequence-half h (rows h*2048 ..
(h+1)*2048). Each core projects K/Q/V only for its OWN 2048 rows, then the
two cores of a batch exchange K^T and V with pair-wise AllGathers (2-core
replica groups), slab-granular so comm hides behind compute.

Precision scheme (validated offline against the seeded reference inputs,
rel-err 1.2e-2 < 2e-2 gate):
  - QKV projections in bf16 (fp8 projections would push score/V noise over
    the error budget).
  - Q^T, K^T, V stored as e4m3 fp8 at scale 16; the scores matmul and the
    attention*V matmul run in fp8 DoubleRow perf mode (256-row contraction
    per instruction, 2x bf16 FLOP rate).
  - P = exp(s) has ~2.7% fp8 quantization noise, too much to hit the error
    budget directly; instead store P' = (P - C)*16 in fp8 (C ~ E[P]) -- a
    3x smaller quantization target -- and add back the rank-1 correction
    C * colsum(V) via a K=1 bf16 matmul opening each output PSUM group.
    colsum(V) over the full key range comes from the *unquantized* V path
    (colsum(x) @ Wv + S*bv, via a DVE free-dim reduce + 16 small matmuls +
    a tiny pair AllReduce), which also cancels the mean component of V's
    fp8 quantization error. Denominator = sum_k P'/256 + S*C via a
    DoubleRow ones-column matmul and a scalar add before the reciprocal.

Per-core math:
  x^T (own half) passed pre-transposed bf16 from host: [F=1024, 2048].
  K^T[h,s] = fp8(16*(sum_f Wk[f,h] x^T[f,s] + bk))   (activation scale+bias)
  Q^T likewise, resident in SBUF. V[s,h] = fp8(16*(x@Wv + bv)).
  S^T[k,q] = sum_h K^T[h,k] Q^T[h,q]  (fp8 DoubleRow, 4 instr per 512 q)
  P = exp(S^T/8192) -> bf16;  P' = (P - C)*16 -> fp8 (DVE 2-op, pair-packed)
  out[q,:] = (P'^T V / 256 + C colsum(V)) / (sum_k P'/256 + S*C)
"""

import numpy as np
import ml_dtypes

# bass_utils' trace path imports antenv.axon_hooks, which some images lack;
# provide a no-op fallback so an externally-set BASS_TRACE cannot crash us.
try:
    import antenv.axon_hooks  # noqa: F401
except Exception:  # pragma: no cover
    try:
        import sys as _sys
        import types as _types

        import antenv as _antenv

        _m = _types.ModuleType("antenv.axon_hooks")
        _m.set_axon_ntff_profile_hook = lambda h: None
        _m.get_axon_ntff_profile_hook = lambda: None
        _sys.modules["antenv.axon_hooks"] = _m
        _antenv.axon_hooks = _m
    except Exception:
        pass

import concourse.bass as bass  # noqa: F401  (registers engine types)
import concourse.mybir as mybir
import concourse.tile as tile
from concourse import bacc
from concourse.bass_utils import run_bass_kernel_spmd

BF16 = mybir.dt.bfloat16
F8 = mybir.dt.float8e4
F32 = mybir.dt.float32
AF = mybir.ActivationFunctionType
DR = mybir.MatmulPerfMode.DoubleRow

B, S, F, H = 4, 4096, 1024, 1024
QH = S // 2  # rows owned per core
FC = F // 128  # 8 feature chunks
HC = H // 128  # 8 hidden chunks
KC = S // 128  # 32 key chunks (full sequence)
N_CORES = 8
QS = 16.0  # fp8 scale for q/k/v/p'
CMEAN = 1.0568  # ~ E[exp(score)] for these inputs; any value is *correct*
EXP_SCALE = 1.0 / (32.0 * QS * QS)  # scores psum carries 256x
DEN_ADD = QS * QS * S * CMEAN  # add to osum psum before reciprocal
PAIRS = [[0, 1], [2, 3], [4, 5], [6, 7]]

# key-chunk processing order: slab-0-dependent chunks (cols 0:1024 of each
# half) first, then slab-1 chunks.  k = half*16 + kk, slab = kk//8.
# Adjacent pairs share a gather-group g = slab*2+half and j parity (even,odd),
# which is exactly the DoubleRow pairing used in the AV matmul.
K_ORDER = (
    list(range(0, 8)) + list(range(16, 24)) + list(range(8, 16)) + list(range(24, 32))
)

_NC_CACHE = None


def _build_nc():
    nc = bacc.Bacc("TRN2", target_bir_lowering=False, debug=False)

    xt_ext = nc.declare_dram_parameter("xt", [F, QH], BF16, isOutput=False)
    wq_ext = nc.declare_dram_parameter("wq", [F, H], BF16, isOutput=False)
    wk_ext = nc.declare_dram_parameter("wk", [F, H], BF16, isOutput=False)
    wv_ext = nc.declare_dram_parameter("wv", [F, H], BF16, isOutput=False)
    bqt_ext = nc.declare_dram_parameter("bqt", [128, HC], F32, isOutput=False)
    bkt_ext = nc.declare_dram_parameter("bkt", [128, HC], F32, isOutput=False)
    bv_ext = nc.declare_dram_parameter("bv", [1, H], BF16, isOutput=False)
    bvr_ext = nc.declare_dram_parameter("bvr", [1, H], F32, isOutput=False)
    out_ext = nc.declare_dram_parameter("out", [QH, H], F32, isOutput=True)

    xt_v = xt_ext[:].rearrange("(c p) s -> p c s", p=128)
    wq_v = wq_ext[:].rearrange("(c p) h -> p c h", p=128)
    wk_v = wk_ext[:].rearrange("(c p) h -> p c h", p=128)
    wv_v = wv_ext[:].rearrange("(c p) h -> p c h", p=128)

    with tile.TileContext(nc) as tc:
        with (
            tc.tile_pool(name="const", bufs=1) as constp,
            tc.tile_pool(name="qtres", bufs=1) as qtpool,
            tc.tile_pool(name="spill", bufs=1, space="DRAM") as dramp,
        ):
            ones_lhs = constp.tile([1, 128], BF16, tag="ones_lhs", name="ones_lhs")
            nc.vector.memset(ones_lhs[:], 1.0)
            cones8 = constp.tile([128, 2, 1], F8, tag="cones8", name="cones8")
            nc.vector.memset(cones8[:], QS)
            bqt = constp.tile([128, HC], F32, tag="bqt", name="bqt")
            bkt = constp.tile([128, HC], F32, tag="bkt", name="bkt")
            bv_sb = constp.tile([1, H], BF16, tag="bv", name="bv_sb")
            bvr_sb = constp.tile([1, H], F32, tag="bvr", name="bvr_sb")
            csrow = constp.tile([1, H], BF16, tag="csrow", name="csrow")

            # per-slab own spills + gathered pair buffers (plain Local DRAM)
            kt_own = [
                dramp.tile([HC, 128, 1024], F8, tag=f"kto{s}", name=f"kt_own{s}")
                for s in range(2)
            ]
            v_own = [
                dramp.tile([1024, H], F8, tag=f"vo{s}", name=f"v_own{s}")
                for s in range(2)
            ]
            kt_gath = [
                dramp.tile([2, HC, 128, 1024], F8, tag=f"ktg{s}", name=f"kt_gath{s}")
                for s in range(2)
            ]
            v_gath = [
                dramp.tile([2, 1024, H], F8, tag=f"vg{s}", name=f"v_gath{s}")
                for s in range(2)
            ]
            cs_own = dramp.tile([1, H], F32, tag="cso", name="cs_own")
            cs_full = dramp.tile([1, H], F32, tag="csf", name="cs_full")

            qt_res = qtpool.tile([128, HC, QH], F8, tag="qtres", name="qt_res")
            # g = slab*2 + half; prefetched during phase A's Q projection
            vbig = [
                qtpool.tile([128, 8, H], F8, tag=f"vb{g}", name=f"vbig{g}")
                for g in range(4)
            ]
            ktbig = [
                qtpool.tile([128, HC, 1024], F8, tag=f"kb{g}", name=f"ktbig{g}")
                for g in range(4)
            ]

            def pair_gather(dst, src):
                nc.gpsimd.collective_compute(
                    "AllGather", mybir.AluOpType.bypass, replica_groups=PAIRS,
                    ins=[src.opt()], outs=[dst.opt()],
                )

            # ---------- Phase A: own-half projections in one x^T pass ----------
            with (
                tc.tile_pool(name="wp", bufs=1) as wp,
                tc.tile_pool(name="xp", bufs=2) as xp,
                tc.tile_pool(name="stage", bufs=2) as stp,
                tc.tile_pool(name="csp", bufs=1) as csp,
                tc.tile_pool(name="psA", bufs=4, space="PSUM") as psA,
                tc.tile_pool(name="psC", bufs=2, space="PSUM") as psC,
            ):
                wk_sb = wp.tile([128, FC, H], BF16, tag="wk", name="wk_sb")
                wq_sb = wp.tile([128, FC, H], BF16, tag="wq", name="wq_sb")
                wv_sb = wp.tile([128, FC, H], BF16, tag="wv", name="wv_sb")
                nc.sync.dma_start(wk_sb[:, :, 0:512], wk_v[:, :, 0:512])

                xts_l = []
                for sp in range(QH // 1024):  # 1024-column slabs of own x^T
                    xts = xp.tile([128, FC, 1024], BF16, tag="xts", name=f"xts{sp}")
                    xts_l.append(xts)
                    base = sp * 1024
                    nc.sync.dma_start(xts[:, :, 0:512], xt_v[:, :, base : base + 512])
                    if sp == 0:
                        nc.sync.dma_start(wk_sb[:, :, 512:1024], wk_v[:, :, 512:1024])
                        nc.sync.dma_start(bkt[:], bkt_ext[:])
                        nc.sync.dma_start(bqt[:], bqt_ext[:])
                        nc.sync.dma_start(bv_sb[:], bv_ext[:])
                        nc.sync.dma_start(bvr_sb[:], bvr_ext[:])
                    nc.sync.dma_start(
                        xts[:, :, 512:1024], xt_v[:, :, base + 512 : base + 1024]
                    )
                # wv is first needed after both K slabs (~55us in), wq after V
                # (~110us): issue after the x^T slabs so they don't contend
                # with the startup-critical loads.
                nc.sync.dma_start(wv_sb[:], wv_v)
                nc.sync.dma_start(wq_sb[:], wq_v)

                # K^T both slabs first, so both pair-gathers start early.
                # Key-half-major order: the second x^T half-slab DMA streams
                # behind the first half's 13us of matmuls instead of stalling
                # the per-hh ps0/ps1 interleave at startup.
                for sp in range(QH // 1024):
                    xts = xts_l[sp]
                    for kh in range(2):
                        kq = slice(kh * 512, kh * 512 + 512)
                        kst = stp.tile(
                            [128, HC, 512], F8, tag="kst", name=f"ks{sp}_{kh}"
                        )
                        # hh pairs alternate two PSUM banks: back-to-back
                        # accumulation into one bank costs ~45ns/matmul extra
                        for hp in range(HC // 2):
                            ps0 = psA.tile(
                                [128, 512], F32, tag="psA", name=f"pk0_{sp}_{kh}_{hp}"
                            )
                            ps1 = psA.tile(
                                [128, 512], F32, tag="psA", name=f"pk1_{sp}_{kh}_{hp}"
                            )
                            for f in range(FC):
                                nc.tensor.matmul(
                                    ps0[:],
                                    wk_sb[:, f, (2 * hp) * 128 : (2 * hp + 1) * 128],
                                    xts[:, f, kq],
                                    start=(f == 0), stop=(f == FC - 1),
                                )
                                nc.tensor.matmul(
                                    ps1[:],
                                    wk_sb[:, f, (2 * hp + 1) * 128 : (2 * hp + 2) * 128],
                                    xts[:, f, kq],
                                    start=(f == 0), stop=(f == FC - 1),
                                )
                            nc.scalar.activation(
                                kst[:, 2 * hp, :], ps0[:], AF.Identity,
                                bias=bkt[:, 2 * hp : 2 * hp + 1], scale=QS,
                            )
                            nc.scalar.activation(
                                kst[:, 2 * hp + 1, :], ps1[:], AF.Identity,
                                bias=bkt[:, 2 * hp + 1 : 2 * hp + 2], scale=QS,
                            )
                        nc.sync.dma_start(
                            kt_own[sp][:, :, kq].rearrange("c p q -> p c q"), kst[:]
                        )
                    pair_gather(kt_gath[sp], kt_own[sp])

                # colsum(V_true) over own half: DVE reduce of x^T + 16 small
                # matmuls against Wv + host-precomputed 2048*bv, then pair
                # AllReduce to cover the partner half. Placed before V proj so
                # the AllReduce queues right after the K^T gathers -- the AV
                # correction matmul consumes csrow at the start of phase B.
                cx0 = csp.tile([128, FC, 1], F32, tag="cx0", name="cx0")
                cx1 = csp.tile([128, FC, 1], F32, tag="cx1", name="cx1")
                cx16 = csp.tile([128, FC, 1], BF16, tag="cx16", name="cx16")
                nc.vector.tensor_reduce(
                    cx0[:], xts_l[0][:], mybir.AxisListType.X, mybir.AluOpType.add
                )
                nc.vector.tensor_reduce(
                    cx1[:], xts_l[1][:], mybir.AxisListType.X, mybir.AluOpType.add
                )
                nc.vector.tensor_tensor(
                    cx16[:], cx0[:], cx1[:], mybir.AluOpType.add
                )
                csO = csp.tile([1, H], F32, tag="csO", name="csO")
                for half in range(2):
                    pcs = psC.tile([1, 512], F32, tag="psC", name=f"pcs{half}")
                    for f in range(FC):
                        nc.tensor.matmul(
                            pcs[:], cx16[:, f, :],
                            wv_sb[:, f, half * 512 : half * 512 + 512],
                            start=(f == 0), stop=(f == FC - 1),
                        )
                    nc.vector.tensor_tensor(
                        csO[:, half * 512 : half * 512 + 512], pcs[:],
                        bvr_sb[:, half * 512 : half * 512 + 512], mybir.AluOpType.add,
                    )
                nc.sync.dma_start(cs_own[:], csO[:])
                nc.gpsimd.collective_compute(
                    "AllReduce", mybir.AluOpType.add, replica_groups=PAIRS,
                    ins=[cs_own[:].opt()], outs=[cs_full[:].opt()],
                )
                csF = csp.tile([1, H], F32, tag="csF", name="csF")
                nc.sync.dma_start(csF[:], cs_full[:])
                # csrow = 256*C*colsum, consumed by the K=1 bf16 corr matmul
                nc.vector.tensor_scalar_mul(csrow[:], csF[:], QS * QS * CMEAN)

                # V both slabs (fp8 out at scale QS; bias via ones-row matmul)
                for sp in range(QH // 1024):
                    xts = xts_l[sp]
                    vst = stp.tile([128, 8, H], F8, tag="vst", bufs=2, name=f"vst{sp}")
                    for sc in range(8):
                        ps0 = psA.tile([128, 512], F32, tag="psA", name=f"pv0_{sp}_{sc}")
                        ps1 = psA.tile([128, 512], F32, tag="psA", name=f"pv1_{sp}_{sc}")
                        for f in range(FC):
                            lhs = xts[:, f, sc * 128 : (sc + 1) * 128]
                            nc.tensor.matmul(
                                ps0[:], lhs, wv_sb[:, f, 0:512],
                                start=(f == 0), stop=False,
                            )
                            nc.tensor.matmul(
                                ps1[:], lhs, wv_sb[:, f, 512:1024],
                                start=(f == 0), stop=False,
                            )
                        nc.tensor.matmul(
                            ps0[:], ones_lhs[:], bv_sb[:, 0:512], start=False, stop=True
                        )
                        nc.tensor.matmul(
                            ps1[:], ones_lhs[:], bv_sb[:, 512:1024], start=False, stop=True
                        )
                        nc.vector.tensor_scalar_mul(vst[:, sc, 0:512], ps0[:], QS)
                        nc.vector.tensor_scalar_mul(vst[:, sc, 512:1024], ps1[:], QS)
                    nc.sync.dma_start(
                        v_own[sp][:].rearrange("(c p) h -> p c h", p=128), vst[:]
                    )
                    pair_gather(v_gath[sp], v_own[sp])

                # prefetch gathered K^T/V into phase-B residents while Q proj
                # still runs (kvpool lives in the outer scope)
                for g in range(4):
                    slab, half = g // 2, g % 2
                    nc.sync.dma_start(
                        ktbig[g][:],
                        kt_gath[slab][half].rearrange("c p k -> p c k"),
                    )
                    nc.sync.dma_start(
                        vbig[g][:],
                        v_gath[slab][half].rearrange("(c p) h -> p c h", p=128),
                    )

                # Q^T both slabs -> resident SBUF (fp8 at scale QS)
                for sp in range(QH // 1024):
                    xts = xts_l[sp]
                    base = sp * 1024
                    for hh in range(HC):
                        ps0 = psA.tile([128, 512], F32, tag="psA", name=f"pq0_{sp}_{hh}")
                        ps1 = psA.tile([128, 512], F32, tag="psA", name=f"pq1_{sp}_{hh}")
                        for f in range(FC):
                            lhs = wq_sb[:, f, hh * 128 : (hh + 1) * 128]
                            nc.tensor.matmul(
                                ps0[:], lhs, xts[:, f, 0:512],
                                start=(f == 0), stop=(f == FC - 1),
                            )
                            nc.tensor.matmul(
                                ps1[:], lhs, xts[:, f, 512:1024],
                                start=(f == 0), stop=(f == FC - 1),
                            )
                        bias = bqt[:, hh : hh + 1]
                        nc.scalar.activation(
                            qt_res[:, hh, base : base + 512], ps0[:],
                            AF.Identity, bias=bias, scale=QS,
                        )
                        nc.scalar.activation(
                            qt_res[:, hh, base + 512 : base + 1024], ps1[:],
                            AF.Identity, bias=bias, scale=QS,
                        )

            # ---------- Phase B: attention, 1024 query rows per tile ----------
            with (
                tc.tile_pool(name="expp", bufs=2) as expp,
                tc.tile_pool(name="pexpp", bufs=4) as pexpp,
                tc.tile_pool(name="obp", bufs=3) as obp,
                tc.tile_pool(name="psS", bufs=3, space="PSUM") as psS,
                tc.tile_pool(name="psO", bufs=2, space="PSUM") as psO,
            ):
                for qt in range(QH // 1024):
                    qbase = qt * 1024
                    # scores + exp + P' quant, one key-chunk at a time
                    exps = {}
                    for pos, k in enumerate(K_ORDER):
                        i, par = pos // 2, pos % 2
                        half, kk = k // 16, k % 16
                        slab, kk8 = kk // 8, kk % 8
                        g = slab * 2 + half
                        if par == 0:
                            exps[i] = expp.tile(
                                [128, 2, 1024], F8, tag=f"e{i}", name=f"e{qt}_{i}"
                            )
                        ps0 = psS.tile([128, 512], F32, tag="psS", name=f"pS0_{qt}_{k}")
                        ps1 = psS.tile([128, 512], F32, tag="psS", name=f"pS1_{qt}_{k}")
                        kslice = slice(kk8 * 128, (kk8 + 1) * 128)
                        for hp in range(HC // 2):
                            nc.tensor.matmul(
                                ps0[:], ktbig[g][:, 2 * hp : 2 * hp + 2, kslice],
                                qt_res[:, 2 * hp : 2 * hp + 2, qbase : qbase + 512],
                                start=(hp == 0), stop=(hp == HC // 2 - 1),
                                perf_mode=DR,
                            )
                        for hp in range(HC // 2):
                            nc.tensor.matmul(
                                ps1[:], ktbig[g][:, 2 * hp : 2 * hp + 2, kslice],
                                qt_res[:, 2 * hp : 2 * hp + 2, qbase + 512 : qbase + 1024],
                                start=(hp == 0), stop=(hp == HC // 2 - 1),
                                perf_mode=DR,
                            )
                        pexp = pexpp.tile([128, 1024], BF16, tag="pexp", name=f"px{qt}_{k}")
                        nc.scalar.activation(pexp[:, 0:512], ps0[:], AF.Exp, scale=EXP_SCALE)
                        nc.scalar.activation(pexp[:, 512:1024], ps1[:], AF.Exp, scale=EXP_SCALE)
                        nc.vector.tensor_scalar(
                            exps[i][:, par, :], pexp[:], -CMEAN, QS,
                            mybir.AluOpType.add, mybir.AluOpType.mult,
                        )
                    # AV: fp8 DoubleRow over key-chunk pairs; the K=1 bf16
                    # corr matmul opens each group with C*colsum(V)*256
                    for q1 in range(8):
                        qo = q1 * 128
                        o0 = psO.tile([128, 512], F32, tag="o0", name=f"o0_{qt}_{q1}")
                        o1 = psO.tile([128, 512], F32, tag="o1", name=f"o1_{qt}_{q1}")
                        osum = psO.tile(
                            [128, 1], F32, tag="osum", bufs=1, name=f"os{qt}_{q1}"
                        )
                        nc.tensor.matmul(
                            o0[:], ones_lhs[:], csrow[:, 0:512], start=True, stop=False
                        )
                        nc.tensor.matmul(
                            o1[:], ones_lhs[:], csrow[:, 512:1024], start=True, stop=False
                        )
                        for t in range(16):
                            k0 = K_ORDER[2 * t]
                            half, kk = k0 // 16, k0 % 16
                            slab, j = kk // 8, kk % 8
                            g = slab * 2 + half
                            lhs = exps[t][:, :, qo : qo + 128]
                            first, last = t == 0, t == 15
                            nc.tensor.matmul(
                                osum[:], lhs, cones8[:], start=first, stop=last,
                                perf_mode=DR,
                            )
                            nc.tensor.matmul(
                                o0[:], lhs, vbig[g][:, j : j + 2, 0:512],
                                start=False, stop=last, perf_mode=DR,
                            )
                            nc.tensor.matmul(
                                o1[:], lhs, vbig[g][:, j : j + 2, 512:1024],
                                start=False, stop=last, perf_mode=DR,
                            )
                        den = obp.tile([128, 1], F32, tag="den", name=f"dn{qt}_{q1}")
                        nc.vector.tensor_scalar_add(den[:], osum[:], DEN_ADD)
                        recip = obp.tile([128, 1], F32, tag="recip", name=f"rc{qt}_{q1}")
                        nc.vector.reciprocal(recip[:], den[:])
                        outsb = obp.tile([128, H], F32, tag="outsb", name=f"ou{qt}_{q1}")
                        row = qbase + qo
                        nc.vector.tensor_scalar_mul(outsb[:, 0:512], o0[:], recip[:])
                        nc.sync.dma_start(
                            out_ext[row : row + 128, 0:512], outsb[:, 0:512]
                        )
                        nc.vector.tensor_scalar_mul(outsb[:, 512:1024], o1[:], recip[:])
                        nc.sync.dma_start(
                            out_ext[row : row + 128, 512:1024], outsb[:, 512:1024]
                        )

    nc.compile()
    return nc


def _get_nc():
    global _NC_CACHE
    if _NC_CACHE is None:
        _NC_CACHE = _build_nc()
    return _NC_CACHE


def _make_in_maps(x, Wq, bq, Wk, bk, Wv, bv):
    bf16 = ml_dtypes.bfloat16
    wq_b = np.asarray(Wq, np.float32).astype(bf16)
    wk_b = np.asarray(Wk, np.float32).astype(bf16)
    wv_b = np.asarray(Wv, np.float32).astype(bf16)
    # activation computes f(scale*x + bias) with scale=QS, so pre-scale biases
    bqt = np.ascontiguousarray(QS * np.asarray(bq, np.float32).reshape(HC, 128).T)
    bkt = np.ascontiguousarray(QS * np.asarray(bk, np.float32).reshape(HC, 128).T)
    bv_b = np.asarray(bv, np.float32).astype(bf16).reshape(1, H)
    bvr = (QH * np.asarray(bv, np.float32)).reshape(1, H)
    x = np.asarray(x, np.float32)
    in_maps = []
    for core in range(N_CORES):
        b, h = core // 2, core % 2
        xt = np.ascontiguousarray(x[b, h * QH : (h + 1) * QH].T).astype(bf16)
        in_maps.append(
            {
                "xt": xt,
                "wq": wq_b,
                "wk": wk_b,
                "wv": wv_b,
                "bqt": bqt,
                "bkt": bkt,
                "bv": bv_b,
                "bvr": bvr,
            }
        )
    return in_maps


def run_on_hw(inputs, trace=False, tmpdir=None):
    """Returns (full_output, BassKernelResults)."""
    nc = _get_nc()
    in_maps = _make_in_maps(**inputs)
    res = run_bass_kernel_spmd(
        nc, in_maps, core_ids=list(range(N_CORES)), trace=trace, tmpdir=tmpdir
    )
    out = np.empty((B, S, H), np.float32)
    for core in range(N_CORES):
        b, h = core // 2, core % 2
        out[b, h * QH : (h + 1) * QH] = res.results[core]["out"]
    return out, res


def kernel(x, Wq, bq, Wk, bk, Wv, bv):
    out, _ = run_on_hw(
        {"x": x, "Wq": Wq, "bq": bq, "Wk": Wk, "bk": bk, "Wv": Wv, "bv": bv}
    )
    return out
